# revision 1
# baseline (speedup 1.0000x reference)
"""GAT (2-layer, 8-head then 1-head) Bass/Tile kernel for Trainium2, 8 NeuronCores.

Sharding: nodes are sorted by in-degree and dealt round-robin to the 8 cores, so
every core sees a nearly identical degree profile and edge count.  Each core
owns the incoming edges of its nodes, laid out densely as
[dst-node-partition x degree-slot]; softmax denominators and weighted feature
sums are then plain free-dim reductions (no scatter / segment ops on device).
Per-edge source-node features are fetched with dma_gather from a replicated
node table (bf16 rows, packed two nodes per 512B row so the int16 gather index
is the pair id; a parity-predicated copy picks the right half).  The attention
projections (a_src/a_dst) are folded into the dense weight matmuls, so the
gathered row already carries [h | a_src | a_dst].  An AllGather shares each
layer's node table between cores.
"""

import os
import sys

import numpy as np

for _p in ("/opt/trn_rl_repo",):
    if _p not in sys.path:
        sys.path.insert(0, _p)

import concourse.bacc as bacc
import concourse.bass as bass
import concourse.mybir as mybir
import concourse.tile as tile
from concourse import bass2jax as _bass2jax
from concourse.bass_utils import run_bass_kernel_spmd

# surface compile-hook exceptions (PJRT swallows the python traceback)
if not getattr(_bass2jax, "_gat_hook_wrapped", False):
    _orig_cc_hook = _bass2jax.neuronx_cc_hook

    def _verbose_cc_hook(*a, **kw):
        try:
            return _orig_cc_hook(*a, **kw)
        except BaseException:
            import traceback

            traceback.print_exc()
            sys.stdout.flush()
            sys.stderr.flush()
            raise

    _bass2jax.neuronx_cc_hook = _verbose_cc_hook
    _bass2jax._gat_hook_wrapped = True
from concourse.masks import make_identity
from contextlib import ExitStack

FP32 = mybir.dt.float32
BF16 = mybir.dt.bfloat16
I16 = mybir.dt.int16
U8 = mybir.dt.uint8

N_CORES = 8
P = 128
NEG_SLOPE = 0.2
EPS = 1e-16

# layer-1: IN=128, HID=8, H=8 ; layer-2: 64 -> 40, 1 head
IN_DIM = 128
H1, C1 = 8, 8
HID1 = H1 * C1  # 64
OUT_DIM = 40

# node-table sub-row layouts (bf16):
#   T1 row: [h(64) | a_src(8) | a_dst(8) | pad(48)]   = 128 bf16 = 256B; pair = 512B
#   T2 row: [h2(40) | a_src2(1) | a_dst2(1) | pad(22)] = 64 bf16 = 128B; pair = 256B
T1_ROW = 128
T2_ROW = int(os.environ.get("GAT_T2ROW", "64"))
T1_HS = 72  # h + a_src contiguous
T2_HS = 41  # h2 + a_src2 contiguous

G_COLS = int(os.environ.get("GAT_GCOLS", "32"))  # gather group size in slot-columns (G_COLS*128 indices per dma_gather)


def _degree_layout(dst, n_nodes, n_cores):
    """Host-side layout: degree-sorted round-robin node assignment plus a
    shared per-block slot-count profile (identical for all cores)."""
    nodes_per_core = -(-n_nodes // (n_cores * P)) * P  # ceil to block multiple
    n_pad = nodes_per_core * n_cores
    deg = np.zeros(n_pad, dtype=np.int64)
    np.add.at(deg, dst, 1)
    order = np.argsort(-deg, kind="stable")  # node ids by degree desc
    rank = np.empty(n_pad, dtype=np.int64)
    rank[order] = np.arange(n_pad)
    # table position of node v: core = rank % n_cores, local = rank // n_cores
    core_of = rank % n_cores
    loc_of = rank // n_cores
    tablepos = core_of * nodes_per_core + loc_of
    n_blocks = nodes_per_core // P
    # per-block slot count: max degree among the block's nodes over all cores
    # == max degree among ranks [j*128*n_cores, (j+1)*128*n_cores)
    deg_by_rank = deg[order]
    d_blocks = []
    for j in range(n_blocks):
        d = int(deg_by_rank[j * P * n_cores : (j + 1) * P * n_cores].max())
        d = max(2, d + (d & 1))  # even, >= 2
        d_blocks.append(d)
    return {
        "nodes_per_core": nodes_per_core,
        "n_pad": n_pad,
        "deg": deg,
        "order": order,
        "core_of": core_of,
        "loc_of": loc_of,
        "tablepos": tablepos,
        "n_blocks": n_blocks,
        "d_blocks": d_blocks,
    }


def _edge_streams(src, dst, lay, n_cores):
    """Build per-core gather-index / parity / pad-mask streams."""
    npc = lay["nodes_per_core"]
    d_blocks = lay["d_blocks"]
    n_blocks = lay["n_blocks"]
    c_total = int(np.sum(d_blocks))
    s_total = c_total * P
    col0 = np.concatenate([[0], np.cumsum(d_blocks)])

    core_of, loc_of, tablepos = lay["core_of"], lay["loc_of"], lay["tablepos"]
    # order edges by destination core / local node
    e_core = core_of[dst]
    e_loc = loc_of[dst]
    idx = [np.zeros(s_total, dtype=np.int16) for _ in range(n_cores)]
    par = [np.zeros(s_total, dtype=np.float32) for _ in range(n_cores)]
    msk = [np.zeros(s_total, dtype=np.float32) for _ in range(n_cores)]
    srcpos = tablepos[src]
    e_sort = np.lexsort((e_loc, e_core))
    e_core_s = e_core[e_sort]
    e_loc_s = e_loc[e_sort]
    e_srcpos_s = srcpos[e_sort]
    core_starts = np.searchsorted(e_core_s, np.arange(n_cores + 1))
    for k in range(n_cores):
        a, b = core_starts[k], core_starts[k + 1]
        locs = e_loc_s[a:b]
        sps = e_srcpos_s[a:b]
        # slot index within each node's run (edges already sorted by loc)
        uniq, first = np.unique(locs, return_index=True)
        slot = np.arange(b - a) - first[np.searchsorted(uniq, locs)]
        blk = locs // P
        n_in_blk = locs % P
        pos = (col0[blk] + slot) * P + n_in_blk
        idx[k][pos] = (sps >> 1).astype(np.int16)
        par[k][pos] = (sps & 1).astype(np.float32)
        msk[k][pos] = 1.0
    return {
        "c_total": c_total,
        "s_total": s_total,
        "col0": col0,
        "idx": idx,
        "par": par,
        "msk": msk,
    }


def _wrap_idx(flat):
    """int16 stream -> [128, len/16] wrapped (16-partition wrap, replicated x8)."""
    w = flat.reshape(-1, 16).T  # [16, len/16]
    return np.tile(w, (8, 1)).copy()


def _col_major(flat):
    """per-slot stream -> [128, n_cols]; position p = col*128 + partition."""
    return np.ascontiguousarray(flat.reshape(-1, P).T)


def _bf16(x):
    import ml_dtypes

    return x.astype(ml_dtypes.bfloat16)


def _build_program(n_cores, npc, d_blocks, c_total, table_rows):
    """Emit the SPMD Bass/Tile program (identical for every core)."""
    nc = bacc.Bacc(
        "TRN2",
        target_bir_lowering=False,
        debug=False,
        num_devices=n_cores,
        num_swdge_queues=int(os.environ.get("GAT_NSWQ", "4")),
    )
    n_blocks = len(d_blocks)
    s_total = c_total * P
    pairs = table_rows // 2

    # --- I/O ---
    xT = nc.dram_tensor("xT", [IN_DIM, npc], BF16, kind="ExternalInput")
    wcat = nc.dram_tensor("wcat", [IN_DIM, 80], BF16, kind="ExternalInput")
    w2cat = nc.dram_tensor("w2cat", [HID1, 42], BF16, kind="ExternalInput")
    bias1 = nc.dram_tensor("bias1", [P, HID1], FP32, kind="ExternalInput")
    bias2 = nc.dram_tensor("bias2", [P, OUT_DIM], FP32, kind="ExternalInput")
    idx16 = nc.dram_tensor("idx16", [P, s_total // 16], I16, kind="ExternalInput")
    par_d = nc.dram_tensor("par", [P, c_total], U8, kind="ExternalInput")
    msk_d = nc.dram_tensor("msk", [P, c_total], BF16, kind="ExternalInput")
    out2 = nc.dram_tensor("out2", [npc, OUT_DIM], FP32, kind="ExternalOutput")

    t1_loc = nc.dram_tensor("t1_loc", [npc, T1_ROW], BF16)
    t2_loc = nc.dram_tensor("t2_loc", [npc, T2_ROW], BF16)
    t1_full = nc.dram_tensor("t1_full", [table_rows, T1_ROW], BF16, addr_space="Shared")
    t2_full = nc.dram_tensor("t2_full", [table_rows, T2_ROW], BF16, addr_space="Shared")

    phases = os.environ.get("GAT_PHASES", "ABC")
    depth = int(os.environ.get("GAT_DEPTH", "9"))
    groups = [(g, min(G_COLS, c_total - g * G_COLS)) for g in range(-(-c_total // G_COLS))]
    col0 = np.concatenate([[0], np.cumsum(d_blocks)])
    replica = [list(range(n_cores))]

    # chunks of consecutive equal-d blocks, capped at CHUNK_COLS slot-columns,
    # so per-chunk reductions are uniform-width and per-block op counts amortize
    CHUNK_COLS = int(os.environ.get("GAT_CHUNK", "64"))
    chunks = []  # (j0, nb, d, c_lo)
    j = 0
    while j < n_blocks:
        d = d_blocks[j]
        j1 = j
        while j1 < n_blocks and d_blocks[j1] == d and (j1 - j + 1) * d <= CHUNK_COLS:
            j1 += 1
        chunks.append((j, j1 - j, d, int(col0[j])))
        j = j1

    with tile.TileContext(nc) as tc, ExitStack() as ctx:
        consts = ctx.enter_context(tc.tile_pool(name="consts", bufs=1))
        wcat_s = consts.tile([IN_DIM, 80], BF16)
        nc.sync.dma_start(wcat_s[:], wcat[:])
        w2cat_s = consts.tile([HID1, 42], BF16)
        nc.sync.dma_start(w2cat_s[:], w2cat[:])
        b1_s = consts.tile([P, HID1], FP32)
        nc.sync.dma_start(b1_s[:], bias1[:])
        b2_s = consts.tile([P, OUT_DIM], FP32)
        nc.sync.dma_start(b2_s[:], bias2[:])
        ident = consts.tile([P, P], BF16)
        make_identity(nc, ident[:])
        # per-block a_dst columns kept on-chip from the producing phase
        adst1_s = consts.tile([P, n_blocks * H1], BF16)
        adst2_s = consts.tile([P, n_blocks], BF16)
        idx_s = consts.tile([P, s_total // 16], I16)
        nc.sync.dma_start(idx_s[:], idx16[:])
        par_s = consts.tile([P, c_total], U8)
        nc.sync.dma_start(par_s[:], par_d[:])
        msk_s = consts.tile([P, c_total], BF16)
        nc.sync.dma_start(msk_s[:], msk_d[:])

        def emit_phases():
            # ---------- phase A: node table T1 = [h | a_src | a_dst | 0] ----------
            BGRP = 6  # blocks per PSUM bank (6*80*4B = 1920B < 2KB)
            with (
                tc.tile_pool(name="pa_sb", bufs=1) as pa_sb,
                tc.tile_pool(name="pa_ps", bufs=2, space="PSUM") as pa_ps,
            ):
                if "A" in phases:
                    xs = pa_sb.tile([IN_DIM, npc], BF16)
                    nc.sync.dma_start(xs[:], xT[:])
                    t1sb = pa_sb.tile([P, n_blocks, T1_ROW], BF16)
                    for g0 in range(0, n_blocks, BGRP):
                        nb = min(BGRP, n_blocks - g0)
                        ps = pa_ps.tile([P, nb * 80], FP32, tag="ps")
                        for b in range(nb):
                            jj = g0 + b
                            nc.tensor.matmul(
                                ps[:, b * 80 : (b + 1) * 80],
                                lhsT=xs[:, jj * P : (jj + 1) * P],
                                rhs=wcat_s[:],
                                start=True,
                                stop=True,
                            )
                        psv = ps[:].rearrange("p (b f) -> p b f", b=nb)
                        nc.vector.tensor_copy(t1sb[:, g0 : g0 + nb, 0:80], psv)
                        nc.vector.tensor_copy(
                            adst1_s[:, g0 * H1 : (g0 + nb) * H1].rearrange(
                                "p (b h) -> p b h", h=H1
                            ),
                            psv[:, :, 72:80],
                        )
                    nc.vector.memset(t1sb[:, :, 80:T1_ROW], 0)
                    nc.sync.dma_start(
                        t1_loc[:].rearrange("(j p) f -> p j f", p=P), t1sb[:]
                    )
            if not os.environ.get("GAT_NOCC"):
                nc.gpsimd.collective_compute(
                    "AllGather",
                    mybir.AluOpType.bypass,
                    replica_groups=replica,
                    ins=[t1_loc[:]],
                    outs=[t1_full[:]],
                )

            # ---------- phase B: layer-1 edges + build T2 ----------
            t1v = t1_full[:].rearrange("(a b) c -> a (b c)", b=2)  # [pairs, 256]

            nswq = int(os.environ.get("GAT_NSWQ", "4"))

            def gather_phase(tv, elem, gb_pool, tag):
                tiles = {}
                for g, cols in groups:
                    gb = gb_pool.tile([P, cols, elem], BF16, tag=tag)
                    nc.gpsimd.dma_gather(
                        out_ap=gb[:],
                        in_ap=tv,
                        idxs_ap=idx_s[:, g * (G_COLS * 8) : g * (G_COLS * 8) + cols * 8],
                        num_idxs=cols * P,
                        num_idxs_reg=cols * P,
                        elem_size=elem,
                        elem_step=elem,
                        single_packet=bool(os.environ.get("GAT_SP")),
                        queue_num=g % nswq,
                    )
                    tiles[g] = gb
                return tiles

            def segments_range(c_lo, c_hi):
                """slot-columns [c_lo, c_hi) -> (group, local-col0, ncols, chunk-col0)."""
                segs = []
                g0, g1 = c_lo // G_COLS, (c_hi - 1) // G_COLS
                for g in range(g0, g1 + 1):
                    lo = max(c_lo, g * G_COLS)
                    hi = min(c_hi, (g + 1) * G_COLS)
                    segs.append((g, lo - g * G_COLS, hi - lo, lo - c_lo))
                return segs

            def edge_chunk(gb_tiles, elem, hs, heads, ch, adst_b, j0, nb, d, c_lo,
                           sel_pool, sm_pool):
                """Process nb consecutive blocks sharing slot width d.
                Returns o1 [P, ch, nb] fp32 (softmax-weighted feature sums)."""
                ncols = nb * d
                if depth < 1:
                    return None
                # feature-major select buffer [P, feature, slot]: unit slot
                # strides downstream enable bf16 2x
                sel_t = sel_pool.tile([P, hs, ncols], BF16, tag="sel")
                selsplit = os.environ.get("GAT_SELSPLIT")
                for si, (g, lc0, ncol, bc0) in enumerate(
                    segments_range(c_lo, c_lo + ncols)
                ):
                    gb = gb_tiles[g]
                    if not os.environ.get("GAT_NOSEL"):
                        src = gb[:, lc0 : lc0 + ncol, 0:hs].transpose([0, 2, 1])
                        dst = sel_t[:, 0:hs, bc0 : bc0 + ncol]
                        if selsplit and si % 2:
                            nc.gpsimd.tensor_copy(dst, src)
                        else:
                            nc.scalar.copy(dst, src)
                    else:
                        nc.vector.memset(sel_t[:, 0:hs, bc0 : bc0 + ncol], 0)
                    if not os.environ.get("GAT_NOPRED"):
                        mask = (
                            par_s[:, c_lo + bc0 : c_lo + bc0 + ncol]
                            .unsqueeze(1)
                            .to_broadcast([P, hs, ncol])
                        )
                        nc.vector.copy_predicated(
                            sel_t[:, 0:hs, bc0 : bc0 + ncol],
                            mask,
                            gb[:, lc0 : lc0 + ncol, elem // 2 : elem // 2 + hs].transpose(
                                [0, 2, 1]
                            ),
                        )
                if depth < 2:
                    return None
                e_t = sm_pool.tile([P, heads, ncols], BF16, tag="e_t")
                a_src = sel_t[:, ch : ch + heads, :].rearrange(
                    "p h (b s) -> p h b s", b=nb
                )
                nc.vector.tensor_tensor(
                    out=e_t[:].rearrange("p h (b s) -> p h b s", b=nb),
                    in0=a_src,
                    in1=adst_b,
                    op=mybir.AluOpType.add,
                )
                e2_t = sm_pool.tile([P, heads, ncols], BF16, tag="e2_t")
                nc.vector.scalar_tensor_tensor(
                    out=e2_t[:],
                    in0=e_t[:],
                    scalar=NEG_SLOPE,
                    op0=mybir.AluOpType.mult,
                    in1=e_t[:],
                    op1=mybir.AluOpType.max,
                )
                ex_t = sm_pool.tile([P, heads, ncols], BF16, tag="ex_t")
                nc.scalar.activation(ex_t[:], e2_t[:], mybir.ActivationFunctionType.Exp)
                exm_t = sm_pool.tile([P, heads, ncols], BF16, tag="exm_t")
                pmask = (
                    msk_s[:, c_lo : c_lo + ncols]
                    .unsqueeze(1)
                    .to_broadcast([P, heads, ncols])
                )
                nc.vector.tensor_tensor(
                    out=exm_t[:], in0=ex_t[:], in1=pmask, op=mybir.AluOpType.mult
                )
                dn = sm_pool.tile([P, heads, nb], FP32, tag="dn")
                nc.vector.reduce_sum(
                    dn[:],
                    exm_t[:].rearrange("p h (b s) -> p h b s", b=nb),
                    axis=mybir.AxisListType.X,
                )
                dne = sm_pool.tile([P, heads, nb], FP32, tag="dne")
                nc.vector.tensor_scalar_add(dne[:], dn[:], EPS)
                rc = sm_pool.tile([P, heads, nb], FP32, tag="rc")
                nc.vector.reciprocal(rc[:], dne[:])
                wm = sm_pool.tile([P, ch, ncols], BF16, tag="wm")
                h_sel = sel_t[:, 0:ch, :].rearrange("p (h c) s -> p h c s", h=heads)
                ex_b = exm_t[:].unsqueeze(2).to_broadcast(
                    [P, heads, ch // heads, ncols]
                )
                wm_4d = wm[:].rearrange("p (h c) s -> p h c s", h=heads)
                nc.vector.tensor_tensor(
                    out=wm_4d, in0=h_sel, in1=ex_b, op=mybir.AluOpType.mult
                )
                ft = sm_pool.tile([P, ch, nb], FP32, tag="ft")
                nc.vector.reduce_sum(
                    ft[:],
                    wm[:].rearrange("p c (b s) -> p c b s", b=nb),
                    axis=mybir.AxisListType.X,
                )
                o1 = sm_pool.tile([P, ch, nb], FP32, tag="o1")
                rc_b = rc[:].unsqueeze(2).to_broadcast([P, heads, ch // heads, nb])
                nc.vector.tensor_tensor(
                    out=o1[:].rearrange("p (h c) b -> p h c b", h=heads),
                    in0=ft[:].rearrange("p (h c) b -> p h c b", h=heads),
                    in1=rc_b,
                    op=mybir.AluOpType.mult,
                )
                return o1

            gbufs = int(os.environ.get("GAT_GBUFS", "6"))
            with (
                tc.tile_pool(name="pb_gb", bufs=gbufs) as pb_gb,
                tc.tile_pool(name="pb_sel", bufs=2) as pb_sel,
                tc.tile_pool(name="pb_sm", bufs=2) as pb_sm,
                tc.tile_pool(name="pb_out", bufs=1) as pb_out,
                tc.tile_pool(name="pb_ps", bufs=2, space="PSUM") as pb_ps,
                tc.tile_pool(name="pb_ps2", bufs=2, space="PSUM") as pb_ps2,
            ):
                gb1 = gather_phase(t1v, 256, pb_gb, "gb1") if "B" in phases else None
                t2sb = pb_out.tile([P, n_blocks, T2_ROW], BF16, name="t2sb", tag="t2sb") if "B" in phases else None
                for j0, nb, d, c_lo in (chunks if "B" in phases else []):
                    adst_b = (
                        adst1_s[:, j0 * H1 : (j0 + nb) * H1]
                        .rearrange("p (b h) -> p b h", h=H1)
                        .transpose([0, 2, 1])
                        .unsqueeze(3)
                        .to_broadcast([P, H1, nb, d])
                    )
                    o1 = edge_chunk(gb1, 256, T1_HS, H1, HID1, adst_b, j0, nb, d,
                                    c_lo, pb_sel, pb_sm)
                    if depth < 3:
                        continue
                    # + bias1, ELU, project to T2 rows
                    o1b = pb_sm.tile([P, HID1, nb], FP32, tag="o1b")
                    nc.vector.tensor_tensor(
                        out=o1b[:],
                        in0=o1[:],
                        in1=b1_s[:].unsqueeze(2).to_broadcast([P, HID1, nb]),
                        op=mybir.AluOpType.add,
                    )
                    mn = pb_sm.tile([P, HID1, nb], FP32, tag="mn")
                    nc.vector.tensor_scalar_min(mn[:], o1b[:], 0.0)
                    exn = pb_sm.tile([P, HID1, nb], FP32, tag="exn")
                    nc.scalar.activation(exn[:], mn[:], mybir.ActivationFunctionType.Exp)
                    rl = pb_sm.tile([P, HID1, nb], FP32, tag="rl")
                    nc.vector.tensor_scalar_max(rl[:], o1b[:], 0.0)
                    # h1c: block-major [P, nb, 64] so per-block slices are unit
                    h1c = pb_sm.tile([P, nb, HID1], BF16, tag="h1c")
                    nc.vector.scalar_tensor_tensor(
                        out=h1c[:].transpose([0, 2, 1]),
                        in0=exn[:],
                        scalar=-1.0,
                        op0=mybir.AluOpType.add,
                        in1=rl[:],
                        op1=mybir.AluOpType.add,
                    )
                    ps2 = pb_ps2.tile([P, nb * 42], FP32, tag="ps2")
                    for b in range(nb):
                        tp = pb_ps.tile([HID1, P], BF16, tag="tp")
                        nc.tensor.transpose(
                            out=tp[:], in_=h1c[:, b, :], identity=ident[:]
                        )
                        h1T = pb_sm.tile([HID1, P], BF16, tag="h1T")
                        nc.vector.tensor_copy(h1T[:], tp[:])
                        nc.tensor.matmul(
                            ps2[:, b * 42 : (b + 1) * 42],
                            lhsT=h1T[:],
                            rhs=w2cat_s[:],
                            start=True,
                            stop=True,
                        )
                    ps2v = ps2[:].rearrange("p (b f) -> p b f", b=nb)
                    nc.vector.tensor_copy(t2sb[:, j0 : j0 + nb, 0:42], ps2v)
                    nc.vector.tensor_copy(
                        adst2_s[:, j0 : j0 + nb].unsqueeze(2), ps2v[:, :, 41:42]
                    )
                if "B" in phases and depth >= 3:
                    nc.vector.memset(t2sb[:, :, 42:T2_ROW], 0)
                    nc.sync.dma_start(
                        t2_loc[:].rearrange("(j p) f -> p j f", p=P), t2sb[:]
                    )
            if not os.environ.get("GAT_NOCC"):
                nc.gpsimd.collective_compute(
                    "AllGather",
                    mybir.AluOpType.bypass,
                    replica_groups=replica,
                    ins=[t2_loc[:]],
                    outs=[t2_full[:]],
                )

            # ---------- phase C: layer-2 edges ----------
            t2v = t2_full[:].rearrange("(a b) c -> a (b c)", b=2)  # [pairs, 2*T2_ROW]
            gbufs2 = int(os.environ.get("GAT_GBUFS2", "8"))
            with (
                tc.tile_pool(name="pc_gb", bufs=gbufs2) as pc_gb,
                tc.tile_pool(name="pc_sel", bufs=2) as pc_sel,
                tc.tile_pool(name="pc_sm", bufs=2) as pc_sm,
                tc.tile_pool(name="pc_out", bufs=1) as pc_out,
            ):
                gb2 = gather_phase(t2v, 2 * T2_ROW, pc_gb, "gb2") if "C" in phases else None
                o2sb = pc_out.tile([P, n_blocks, OUT_DIM], FP32, name="o2sb", tag="o2sb") if "C" in phases else None
                for j0, nb, d, c_lo in (chunks if "C" in phases else []):
                    adst_b = (
                        adst2_s[:, j0 : j0 + nb]
                        .unsqueeze(1)
                        .unsqueeze(3)
                        .to_broadcast([P, 1, nb, d])
                    )
                    o2 = edge_chunk(gb2, 2 * T2_ROW, T2_HS, 1, OUT_DIM, adst_b, j0,
                                    nb, d, c_lo, pc_sel, pc_sm)
                    if depth < 4:
                        continue
                    nc.vector.tensor_tensor(
                        out=o2sb[:, j0 : j0 + nb, :].transpose([0, 2, 1]),
                        in0=o2[:],
                        in1=b2_s[:].unsqueeze(2).to_broadcast([P, OUT_DIM, nb]),
                        op=mybir.AluOpType.add,
                    )
                if "C" in phases and depth >= 4:
                    nc.sync.dma_start(
                        out2[:].rearrange("(j p) f -> p j f", p=P), o2sb[:]
                    )

        for _rep in range(int(os.environ.get("GAT_REPEAT", "1"))):
            emit_phases()

    nc.compile()
    return nc


_CACHE = {}
LAST_RESULTS = None
LAST_EXEC_S = None


def kernel(**inputs) -> np.ndarray:
    x = np.asarray(inputs["x"], dtype=np.float32)
    edge_index = np.asarray(inputs["edge_index"])
    W1 = np.asarray(inputs["W1"], dtype=np.float32)
    att_src1 = np.asarray(inputs["att_src1"], dtype=np.float32)
    att_dst1 = np.asarray(inputs["att_dst1"], dtype=np.float32)
    b1 = np.asarray(inputs["bias1"], dtype=np.float32)
    W2 = np.asarray(inputs["W2"], dtype=np.float32)
    att_src2 = np.asarray(inputs["att_src2"], dtype=np.float32)
    att_dst2 = np.asarray(inputs["att_dst2"], dtype=np.float32)
    b2 = np.asarray(inputs["bias2"], dtype=np.float32)

    n_nodes = x.shape[0]
    src = np.asarray(edge_index[0], dtype=np.int64)
    dst = np.asarray(edge_index[1], dtype=np.int64)

    lay = _degree_layout(dst, n_nodes, N_CORES)
    streams = _edge_streams(src, dst, lay, N_CORES)
    npc = lay["nodes_per_core"]
    table_rows = lay["n_pad"]

    key = (npc, tuple(lay["d_blocks"]), streams["c_total"], table_rows)
    if key not in _CACHE:
        _CACHE[key] = _build_program(
            N_CORES, npc, lay["d_blocks"], streams["c_total"], table_rows
        )
    nc = _CACHE[key]

    # host-side parameter folding
    a_src1 = np.stack(
        [W1[:, h * C1 : (h + 1) * C1] @ att_src1[h] for h in range(H1)], axis=1
    )  # [128, 8]
    a_dst1 = np.stack(
        [W1[:, h * C1 : (h + 1) * C1] @ att_dst1[h] for h in range(H1)], axis=1
    )
    wcat = _bf16(np.concatenate([W1, a_src1, a_dst1], axis=1))  # [128, 80]
    w2cat = _bf16(
        np.concatenate([W2, W2 @ att_src2[0][:, None], W2 @ att_dst2[0][:, None]], axis=1)
    )  # [64, 42]

    # per-core inputs
    in_maps = []
    x_pad = np.zeros((lay["n_pad"], IN_DIM), dtype=np.float32)
    x_pad[:n_nodes] = x
    for k in range(N_CORES):
        ranks = np.arange(k, lay["n_pad"], N_CORES)
        node_ids = lay["order"][ranks]
        xk = x_pad[node_ids]  # [npc, 128]
        in_maps.append(
            {
                "xT": np.ascontiguousarray(_bf16(xk.T)),
                "wcat": wcat,
                "w2cat": w2cat,
                "bias1": np.tile(b1.reshape(1, -1), (P, 1)),
                "bias2": np.tile(b2.reshape(1, -1), (P, 1)),
                "idx16": _wrap_idx(
                    streams["idx"][k] * 0
                    if os.environ.get("GAT_ZIDX")
                    else streams["idx"][k]
                ),
                "par": _col_major(streams["par"][k]).astype(np.uint8),
                "msk": _bf16(_col_major(streams["msk"][k])),
            }
        )

    if os.environ.get("GAT_BASS_SIM"):
        from concourse.bass_interp import MultiCoreSim

        sim = MultiCoreSim(nc, num_cores=N_CORES, trace=False)
        for k in range(N_CORES):
            for name, arr in in_maps[k].items():
                sim.cores[k].tensor(name)[:] = arr
        sim.simulate(check_with_hw=False)
        results = [{"out2": np.array(sim.cores[k].tensor("out2"))} for k in range(N_CORES)]
    else:
        import time as _time

        _t0 = _time.time()
        res = run_bass_kernel_spmd(
            nc,
            in_maps,
            list(range(N_CORES)),
            trace=bool(os.environ.get("GAT_BASS_TRACE")),
        )
        global LAST_RESULTS, LAST_EXEC_S
        LAST_EXEC_S = _time.time() - _t0
        results = res.results
        LAST_RESULTS = res

    out = np.zeros((n_nodes, OUT_DIM), dtype=np.float32)
    for k in range(N_CORES):
        ranks = np.arange(k, lay["n_pad"], N_CORES)
        node_ids = lay["order"][ranks]
        ok = results[k]["out2"]
        keep = node_ids < n_nodes
        out[node_ids[keep]] = ok[keep]
    return out

